# revision 2
# baseline (speedup 1.0000x reference)
"""Trainium2 Bass kernel for Coo2FulSimple (periodic pairwise squared
distances + cutoff adjacency mask).

Contract: kernel(**inputs) takes the FULL unsharded inputs (numpy) and
returns the FULL outputs (out [B,N,N,S] f32, mask [B,N,N,S] bool),
matching reference.reference() bit-for-bit.

Device computes the UNMASKED squared-distance tensor sod (bit-exact
IEEE f32, same op order as the jax reference); the host applies the
cutoff mask (exact f32 compare) and the self-pair exclusion.

Sharding + symmetry: sod is symmetric under (i<->j, s->26-s) exactly in
IEEE f32 (vec flips sign elementwise; squares/sums match). Per batch,
only the 10 upper-triangle 128x128 (i-tile, j-tile) blocks are computed
(out of 16); mirrors are filled on the host. 4 batches x 10 blocks = 40
blocks over 8 cores = 5 blocks/core, uniform SPMD program.

Math per block (plane-major SBUF [128 i, plane, 128 j], all flat ops):
  D_c    = (-pos_j[c]) + pos_i[c]          DVE tensor_scalar (1 round)
  V_ck   = 1.0*D_c + t_ck                  ACT Identity bias-add (1 round)
  W_ck   = V_ck^2                          ACT Square (c=0,1) / GpSimd mult (c=2)
  P_g    = W0_k0 + W1_k1                   DVE tensor_tensor (stride-0 views)
  sod_s  = P_{s//3} + W2_{s%3}             DVE/GpSimd tensor_tensor, s-plane-major
DMA out per k0-group of 9 s-planes -> HBM [blk, i, s, j]; host transposes.
"""

import os
from contextlib import ExitStack

import numpy as np

B, N, S = 4, 512, 27
NCORES = 8
IT = 128          # i-tile size == SBUF partitions
JT = 128          # j-tile size
NBLK = 5          # blocks per core
RC2 = 36.0

# (i-tile, j-tile) blocks per core parity; core k -> batch k//2.
BLOCKS_EVEN = [(0, 0), (0, 1), (0, 2), (0, 3), (3, 3)]
BLOCKS_ODD = [(1, 1), (1, 2), (1, 3), (2, 2), (2, 3)]

# sod k0-group -> engine: 'v' (DVE) or 'g' (GpSimd), per block.
# 9 of 15 groups on DVE, 6 on GpSimd (balances measured rates).
SOD_ENG = [
    ("v", "v", "g"),
    ("v", "v", "g"),
    ("v", "v", "g"),
    ("v", "v", "g"),
    ("v", "g", "g"),
]

# cst layout (width CW): per block u: 3*JT cols of -pos_j (c-major),
# then per block u: 3 cols pos_i, then 9 cols tvals.
PJ0 = 0
PI0 = NBLK * 3 * JT          # 1920
TB0 = PI0 + NBLK * 3         # 1935
CW = TB0 + 9                 # 1944

_CACHE = {}


def _build_program():
    import concourse.bacc as bacc
    import concourse.mybir as mybir
    import concourse.tile as tile

    f32 = mybir.dt.float32
    IDENT = mybir.ActivationFunctionType.Identity
    SQUARE = mybir.ActivationFunctionType.Square
    ADD = mybir.AluOpType.add
    MULT = mybir.AluOpType.mult

    nc = bacc.Bacc(
        "TRN2", target_bir_lowering=False, debug=False, num_devices=NCORES
    )

    cst = nc.dram_tensor("cst", [IT, CW], f32, kind="ExternalInput").ap()
    outv = nc.dram_tensor("outv", [NBLK, IT, S, JT], f32, kind="ExternalOutput").ap()

    with ExitStack() as ctx:
        tc = ctx.enter_context(tile.TileContext(nc))
        const = ctx.enter_context(tc.tile_pool(name="const", bufs=1))
        cst_sb = const.tile([IT, CW], f32)
        nc.sync.dma_start(cst_sb[:], cst)

        dpool = ctx.enter_context(tc.tile_pool(name="dpool", bufs=2))
        vpool = ctx.enter_context(tc.tile_pool(name="vpool", bufs=2))
        wpool = ctx.enter_context(tc.tile_pool(name="wpool", bufs=2))
        w2pool = ctx.enter_context(tc.tile_pool(name="w2pool", bufs=2))
        ppool = ctx.enter_context(tc.tile_pool(name="ppool", bufs=2))
        spool = ctx.enter_context(tc.tile_pool(name="spool", bufs=2))

        for u in range(NBLK):
            pj = cst_sb[:, u * 3 * JT : (u + 1) * 3 * JT].rearrange(
                "p (c j) -> p c j", c=3
            )

            # --- DVE: D_c = (-pos_j) + pos_i  (single rounding) ---
            Dt = dpool.tile([IT, 3, JT], f32)
            for c in range(3):
                pic = cst_sb[:, PI0 + 3 * u + c : PI0 + 3 * u + c + 1]
                nc.vector.tensor_scalar(Dt[:, c, :], pj[:, c, :], pic, None, ADD)

            # --- ACT: V_ck = 1.0*D_c + t_ck  (fma, 1 rounding) ---
            Vt = vpool.tile([IT, 9, JT], f32)
            for c in range(3):
                for k in range(3):
                    m = 3 * c + k
                    nc.scalar.activation(
                        Vt[:, m, :],
                        Dt[:, c, :],
                        IDENT,
                        bias=cst_sb[:, TB0 + m : TB0 + m + 1],
                        scale=1.0,
                    )

            # --- ACT: W01 = V01^2 ; GpSimd: W2 = V2*V2 ---
            W01 = wpool.tile([IT, 6, JT], f32)
            nc.scalar.activation(
                W01[:].rearrange("p m j -> p (m j)"),
                Vt[:, 0:6, :].rearrange("p m j -> p (m j)"),
                SQUARE,
            )
            W2s = w2pool.tile([IT, 3, JT], f32)
            nc.gpsimd.tensor_tensor(W2s[:], Vt[:, 6:9, :], Vt[:, 6:9, :], MULT)

            # --- DVE: P_g = W0_k0 + W1_k1 (g = 3*k0+k1), flat out ---
            Pt = ppool.tile([IT, 9, JT], f32)
            w0b = W01[:, 0:3, :].unsqueeze(2).broadcast_to([IT, 3, 3, JT])
            w1b = W01[:, 3:6, :].unsqueeze(1).broadcast_to([IT, 3, 3, JT])
            nc.vector.tensor_tensor(
                Pt[:].rearrange("p (a b) j -> p a b j", a=3), w0b, w1b, ADD
            )

            # --- sod_s = P_{s//3} + W2_{s%3}, one op per k0-group of 9 ---
            sod = spool.tile([IT, S, JT], f32)
            for k0 in range(3):
                pb = (
                    Pt[:, 3 * k0 : 3 * k0 + 3, :]
                    .unsqueeze(2)
                    .broadcast_to([IT, 3, 3, JT])
                )
                w2b = W2s[:].unsqueeze(1).broadcast_to([IT, 3, 3, JT])
                so = sod[:, 9 * k0 : 9 * k0 + 9, :].rearrange(
                    "p (a b) j -> p a b j", a=3
                )
                eng = nc.vector if SOD_ENG[u][k0] == "v" else nc.gpsimd
                eng.tensor_tensor(so, pb, w2b, ADD)
                nc.sync.dma_start(
                    outv[u, :, 9 * k0 : 9 * k0 + 9, :],
                    sod[:, 9 * k0 : 9 * k0 + 9, :],
                )

    nc.compile()
    return nc


def _get_program():
    if "nc" not in _CACHE:
        _CACHE["nc"] = _build_program()
    return _CACHE["nc"]


def _core_blocks(k):
    return BLOCKS_EVEN if k % 2 == 0 else BLOCKS_ODD


def _prep_core_inputs(pos, tvals):
    in_maps = []
    for k in range(NCORES):
        b = k // 2
        cst = np.empty((IT, CW), np.float32)
        for u, (ib, jb) in enumerate(_core_blocks(k)):
            # -pos_j, c-major, replicated over partitions
            cst[:, u * 3 * JT : (u + 1) * 3 * JT] = (
                -pos[b, jb * JT : (jb + 1) * JT, :].T
            ).reshape(1, 3 * JT)
            # pos_i per partition
            cst[:, PI0 + 3 * u : PI0 + 3 * u + 3] = pos[
                b, ib * IT : (ib + 1) * IT, :
            ]
        cst[:, TB0 : TB0 + 9] = tvals.reshape(1, 9)
        in_maps.append({"cst": cst})
    return in_maps


def _gather(results):
    sod = np.empty((B, N, N, S), np.float32)
    for k in range(NCORES):
        b = k // 2
        ov = results[k]["outv"]  # [NBLK, IT, S, JT]
        for u, (ib, jb) in enumerate(_core_blocks(k)):
            blk = ov[u]  # [i, s, j]
            iR = slice(ib * IT, (ib + 1) * IT)
            jR = slice(jb * JT, (jb + 1) * JT)
            sod[b, iR, jR, :] = blk.transpose(0, 2, 1)
            if ib != jb:
                # mirror: sod[j,i,s] = sod[i,j,26-s] (IEEE-exact symmetry)
                sod[b, jR, iR, :] = blk[:, ::-1, :].transpose(2, 0, 1)
    return sod


def _analyze_shifts(cel_mat, sft_cel):
    """Return tvals[9] f32 if inputs have the standard structure
    (diagonal shared cell, sft = meshgrid(-1..1)^3), else None.

    tvals[3*c + k] is the k-th shift value on axis c, so that
    s = 9*k0 + 3*k1 + k2 indexes sft_xyz[s] = (t0[k0], t1[k1], t2[k2]).
    """
    r = np.arange(-1, 2)
    expect = np.stack(np.meshgrid(r, r, r, indexing="ij"), axis=-1).reshape(-1, 3)
    if sft_cel.shape != (27, 3) or not np.array_equal(sft_cel, expect):
        return None
    cel0 = cel_mat[0]
    if not np.all(cel_mat == cel0[None]):
        return None
    if np.any(cel0 != np.diag(np.diag(cel0))):
        return None
    diag = np.diag(cel0).astype(np.float32)
    tvals = np.empty(9, np.float32)
    for c in range(3):
        for k in range(3):
            tvals[3 * c + k] = np.float32(np.float32(k - 1) * diag[c])
    return tvals


def _reference_fallback(pos_xyz, cel_mat, pbc, ent, sft_cel):
    """Plain numpy mirror of the reference (for non-standard inputs only)."""
    sft_xyz = np.einsum("sd,bde->bse", sft_cel.astype(cel_mat.dtype), cel_mat)
    vec = (
        pos_xyz[:, :, None, None, :]
        - pos_xyz[:, None, :, None, :]
        + sft_xyz[:, None, None, :, :]
    )
    sod = np.sum(vec * vec, axis=-1)
    n = pos_xyz.shape[1]
    eye = np.eye(n, dtype=bool)
    zero_sft = np.all(sft_cel == 0, axis=-1)
    self_pair = eye[None, :, :, None] & zero_sft[None, None, None, :]
    val = ent[:, :, None, None] & ent[:, None, :, None]
    mask = (sod <= RC2) & val & ~self_pair
    out = np.where(mask, sod, np.zeros((), sod.dtype))
    return out, mask


def kernel(pos_xyz, cel_mat, pbc, ent, sft_cel):
    pos_xyz = np.asarray(pos_xyz)
    cel_mat = np.asarray(cel_mat)
    pbc = np.asarray(pbc)
    ent = np.asarray(ent)
    sft_cel = np.asarray(sft_cel)

    tvals = None
    if pos_xyz.shape == (B, N, 3) and pos_xyz.dtype == np.float32:
        tvals = _analyze_shifts(cel_mat, sft_cel)
    if tvals is None:
        return _reference_fallback(pos_xyz, cel_mat, pbc, ent, sft_cel)

    from concourse.bass_utils import run_bass_kernel_spmd

    nc = _get_program()
    in_maps = _prep_core_inputs(pos_xyz, tvals)
    trace = os.environ.get("BENCH_TRACE", "") == "1"
    res = run_bass_kernel_spmd(
        nc, in_maps, core_ids=list(range(NCORES)), trace=trace
    )
    _CACHE["last_results"] = res
    sod = _gather(res.results)

    # Host-side masking (exact f32 compare; sod*1.0 and sod*0.0 are exact).
    mask = np.empty((B, N, N, S), np.bool_)
    for b in range(B):
        mb = sod[b] <= RC2
        sod[b] *= mb
        mask[b] = mb
    idx = np.arange(N)
    mask[:, idx, idx, 13] = False  # self pairs (sod there is exactly +0.0)
    out = sod
    if not ent.all():
        val = ent[:, :, None, None] & ent[:, None, :, None]
        mask &= val[..., None]
        out *= mask
    return out, mask
